# revision 5
# baseline (speedup 1.0000x reference)
"""KVCache.fill scatter kernel for Trainium2 (8 NeuronCores, SPMD).

Strategy
--------
The reference scatters k_val/v_val (B,H,S,D) into k_cache/v_cache (B,H,L,D)
at `fill_indices` along the L axis, and input_pos into pos.

Sharding: flatten (B,H) = 128 planes, 16 planes per core (pure data
parallel); fill_indices/input_pos are replicated.

Per plane the output is a row-merge of two sources (cache rows where not
filled, val rows where filled). Because the Bass program is compiled inside
kernel() at call time, the merge pattern is known host-side. The host
stages each core's source in a row-interleaved layout

    src_t[row, plane, d] = concat([cache_rows(L) ; val_rows(S)])

so that ONE dma_gather descriptor (one index) moves the same logical row of
ALL 16 planes: a 4KB contiguous element. The device builds the merged
output with 8 dma_gathers per tensor (1024 indices each, 4KB/descriptor,
HBM->SBUF) + 8 contiguous 4MB stores (SBUF->HBM, 4KB chunks). Descriptor
count per core is 16K (vs 512K at 256B granularity) which keeps the GpSimd
Q7 descriptor-generation off the critical path; every DMA descriptor is
>=4KB so both reads and writes run at line rate.

The device output layout is row-interleaved (L, 16, D); the host transposes
back to (16, L, D) when assembling the full result (pure data staging).

pos (B,1,L) int32 is merged on DVE: pos_new = (pos & keep_mask) | dense,
with keep_mask/dense built host-side from fill_indices/input_pos (index
metadata only). Every core computes it redundantly; core 0's copy is used.
"""

import sys

if "/opt/trn_rl_repo" not in sys.path:
    sys.path.insert(0, "/opt/trn_rl_repo")

import numpy as np

import concourse.bacc as bacc
import concourse.bass as bass
import concourse.mybir as mybir
from concourse.bass_utils import run_bass_kernel_spmd
from concourse.library_config import mlp

B, H, L, D, S = 4, 32, 8192, 128, 2048
N_CORES = 8
PLANES = (B * H) // N_CORES          # 16 planes per core
SRCR = L + S                         # 10240 rows in concat source
EL = PLANES * D                      # 2048 bf16 elements (4KB) per gathered row
NIG = 2048                           # indices per dma_gather
NGATH = L // NIG                     # 8 gathers per tensor
NBUF = 2                             # rotating SBUF tiles

_NC_CACHE = None


def _build_nc():
    nc = bacc.Bacc("TRN2", target_bir_lowering=False)
    src_k = nc.dram_tensor("src_k", [SRCR, EL], mybir.dt.bfloat16, kind="ExternalInput")
    src_v = nc.dram_tensor("src_v", [SRCR, EL], mybir.dt.bfloat16, kind="ExternalInput")
    gidx = nc.dram_tensor("gidx", [128, L // 16], mybir.dt.int16, kind="ExternalInput")
    posw = nc.dram_tensor("posw", [128, B * L // 128], mybir.dt.int32, kind="ExternalInput")
    pmask = nc.dram_tensor("pmask", [128, B * L // 128], mybir.dt.int32, kind="ExternalInput")
    pdense = nc.dram_tensor("pdense", [128, B * L // 128], mybir.dt.int32, kind="ExternalInput")
    out_k = nc.dram_tensor("out_k", [L, EL], mybir.dt.bfloat16, kind="ExternalOutput")
    out_v = nc.dram_tensor("out_v", [L, EL], mybir.dt.bfloat16, kind="ExternalOutput")
    out_pos = nc.dram_tensor("out_pos", [128, B * L // 128], mybir.dt.int32, kind="ExternalOutput")

    pw = B * L // 128  # 256 int32 per partition
    JB = NIG // 128    # 8 column blocks per gather

    from contextlib import ExitStack

    with ExitStack() as ctx:
        block = ctx.enter_context(nc.Block())
        dst = [
            ctx.enter_context(
                nc.sbuf_tensor(f"dst{i}", [128, JB, EL], mybir.dt.bfloat16)
            )
            for i in range(NBUF)
        ]
        idx_sb = ctx.enter_context(nc.sbuf_tensor("idx_sb", [128, L // 16], mybir.dt.int16))
        pos_sb = ctx.enter_context(nc.sbuf_tensor("pos_sb", [128, pw], mybir.dt.int32))
        pm_sb = ctx.enter_context(nc.sbuf_tensor("pm_sb", [128, pw], mybir.dt.int32))
        pd_sb = ctx.enter_context(nc.sbuf_tensor("pd_sb", [128, pw], mybir.dt.int32))
        po_sb = ctx.enter_context(nc.sbuf_tensor("po_sb", [128, pw], mybir.dt.int32))
        ix = ctx.enter_context(nc.semaphore("ix"))
        io = ctx.enter_context(nc.semaphore("io"))
        pv = ctx.enter_context(nc.semaphore("pv"))
        g_sem = [ctx.enter_context(nc.semaphore(f"g{i}")) for i in range(NBUF)]
        s_sem = [ctx.enter_context(nc.semaphore(f"s{i}")) for i in range(NBUF)]

        NT = 2 * NGATH  # total gather/store pairs (k then v)

        def src_of(t):
            return src_k if t < NGATH else src_v

        def out_view(t):
            g = t % NGATH
            out = out_k if t < NGATH else out_v
            return out[g * NIG : (g + 1) * NIG].rearrange("(j p) e -> p j e", p=128)

        def idx_slice(t):
            g = t % NGATH
            return idx_sb[:, g * (NIG // 16) : (g + 1) * (NIG // 16)]

        @block.sync
        def _(sp):
            sp.dma_start(idx_sb[:], gidx[:]).then_inc(ix, 16)
            sp.dma_start(pos_sb[:], posw[:]).then_inc(io, 16)
            sp.dma_start(pm_sb[:], pmask[:]).then_inc(io, 16)
            sp.dma_start(pd_sb[:], pdense[:]).then_inc(io, 16)
            for t in range(NT):
                b, n = t % NBUF, t // NBUF
                sp.wait_ge(g_sem[b], 16 * (n + 1))
                sp.dma_start(out_view(t), dst[b][:]).then_inc(s_sem[b], 16)
            sp.wait_ge(pv, 1)
            sp.dma_start(out_pos[:], po_sb[:]).then_inc(io, 16)
            sp.wait_ge(io, 64)
            for b in range(NBUF):
                sp.wait_ge(s_sem[b], 16 * (NT // NBUF))

        @block.vector
        def _(ve):
            ve.wait_ge(io, 48)  # pos, mask, dense loaded
            ve.tensor_tensor(
                out=po_sb[:], in0=pos_sb[:], in1=pm_sb[:],
                op=mybir.AluOpType.bitwise_and,
            )
            ve.drain()
            ve.tensor_tensor(
                out=po_sb[:], in0=po_sb[:], in1=pd_sb[:],
                op=mybir.AluOpType.bitwise_or,
            ).then_inc(pv, 1)

        @block.gpsimd
        def _(gp):
            gp.load_library(mlp)
            gp.wait_ge(ix, 16)
            for t in range(NT):
                b, n = t % NBUF, t // NBUF
                if n > 0:
                    gp.wait_ge(s_sem[b], 16 * n)
                gp.dma_gather(
                    dst[b][:], src_of(t)[:], idx_slice(t), NIG, NIG, EL,
                    single_packet=False,
                ).then_inc(g_sem[b], 16)

    nc.compile()
    return nc


def _get_nc():
    global _NC_CACHE
    if _NC_CACHE is None:
        _NC_CACHE = _build_nc()
    return _NC_CACHE


def _host_prep(k_cache, v_cache, pos, fill_indices, input_pos, k_val, v_val):
    fi = np.asarray(fill_indices).astype(np.int64)
    ip = np.asarray(input_pos).astype(np.int32)

    # output row -> row in row-interleaved concat source [cache(L) ; val(S)]
    src_of = np.arange(L, dtype=np.int64)
    src_of[fi] = L + np.arange(S)
    idx_flat = src_of.astype(np.int16)                       # natural row order
    gidx = np.tile(idx_flat.reshape(L // 16, 16).T, (8, 1))  # replicated per Q7 core

    kc = np.asarray(k_cache).reshape(B * H, L, D)
    vc = np.asarray(v_cache).reshape(B * H, L, D)
    kv = np.asarray(k_val).reshape(B * H, S, D)
    vv = np.asarray(v_val).reshape(B * H, S, D)

    posw = np.ascontiguousarray(np.asarray(pos).astype(np.int32)).reshape(128, -1)
    keep = np.full(L, -1, np.int32)
    keep[fi] = 0
    dense = np.zeros(L, np.int32)
    dense[fi] = ip
    pmask = np.tile(keep, B).reshape(128, -1)
    pdense = np.tile(dense, B).reshape(128, -1)

    in_maps = []
    for c in range(N_CORES):
        sl = slice(c * PLANES, (c + 1) * PLANES)
        # (PLANES, rows, D) -> (rows, PLANES, D) -> (rows, EL)
        src_k = np.concatenate(
            [kc[sl].transpose(1, 0, 2), kv[sl].transpose(1, 0, 2)], axis=0
        ).reshape(SRCR, EL)
        src_v = np.concatenate(
            [vc[sl].transpose(1, 0, 2), vv[sl].transpose(1, 0, 2)], axis=0
        ).reshape(SRCR, EL)
        in_maps.append(
            {
                "src_k": np.ascontiguousarray(src_k),
                "src_v": np.ascontiguousarray(src_v),
                "gidx": gidx,
                "posw": posw,
                "pmask": pmask,
                "pdense": pdense,
            }
        )
    return in_maps


def _assemble(res):
    k_parts, v_parts = [], []
    for c in range(N_CORES):
        # (L, EL) -> (L, PLANES, D) -> (PLANES, L, D)
        k_parts.append(res.results[c]["out_k"].reshape(L, PLANES, D).transpose(1, 0, 2))
        v_parts.append(res.results[c]["out_v"].reshape(L, PLANES, D).transpose(1, 0, 2))
    k_new = np.concatenate(k_parts).reshape(B, H, L, D)
    v_new = np.concatenate(v_parts).reshape(B, H, L, D)
    pos_new = res.results[0]["out_pos"].reshape(B, 1, L)
    return k_new, v_new, pos_new


def run(inputs, trace=False):
    """Run the device kernel; returns ((k_new, v_new, pos_new), BassKernelResults)."""
    nc = _get_nc()
    in_maps = _host_prep(**inputs)
    res = run_bass_kernel_spmd(
        nc, in_maps, core_ids=list(range(N_CORES)), trace=trace
    )
    return _assemble(res), res


def kernel(**inputs):
    out, _ = run(inputs, trace=False)
    return out


# revision 6
# speedup vs baseline: 1.1108x; 1.1108x over previous
"""KVCache.fill scatter kernel for Trainium2 (8 NeuronCores, SPMD).

Strategy
--------
The reference scatters k_val/v_val (B,H,S,D) into k_cache/v_cache (B,H,L,D)
at `fill_indices` along the L axis, and input_pos into pos.

Sharding: flatten (B,H) = 128 planes, 16 planes per core (pure data
parallel); fill_indices/input_pos are replicated.

Per plane the output is a row-merge of two sources (cache rows where not
filled, val rows where filled). Because the Bass program is compiled inside
kernel() at call time, the merge pattern is known host-side. The host
stages each core's source in a row-interleaved layout

    src_t[row, plane, d] = concat([cache_rows(L) ; val_rows(S)])

so that ONE dma_gather descriptor (one index) moves the same logical row of
ALL 16 planes: a 4KB contiguous element. The device builds the merged
output with 8 dma_gathers per tensor (1024 indices each, 4KB/descriptor,
HBM->SBUF) + 8 contiguous 4MB stores (SBUF->HBM, 4KB chunks). Descriptor
count per core is 16K (vs 512K at 256B granularity) which keeps the GpSimd
Q7 descriptor-generation off the critical path; every DMA descriptor is
>=4KB so both reads and writes run at line rate.

The device output layout is row-interleaved (L, 16, D); the host transposes
back to (16, L, D) when assembling the full result (pure data staging).

pos (B,1,L) int32 is merged on DVE: pos_new = (pos & keep_mask) | dense,
with keep_mask/dense built host-side from fill_indices/input_pos (index
metadata only). Every core computes it redundantly; core 0's copy is used.
"""

import sys

if "/opt/trn_rl_repo" not in sys.path:
    sys.path.insert(0, "/opt/trn_rl_repo")

import numpy as np

import concourse.bacc as bacc
import concourse.bass as bass
import concourse.mybir as mybir
from concourse.bass_utils import run_bass_kernel_spmd
from concourse.library_config import mlp

B, H, L, D, S = 4, 32, 8192, 128, 2048
N_CORES = 8
PLANES = (B * H) // N_CORES          # 16 planes per core
SRCR = L + S                         # 10240 rows in concat source
EL = PLANES * D                      # 2048 bf16 elements (4KB) per gathered row
NIG = 1024                           # indices per dma_gather
NGATH = L // NIG                     # 8 gathers per tensor
NBUF = 4                             # rotating SBUF tiles

_NC_CACHE = None


def _build_nc():
    nc = bacc.Bacc("TRN2", target_bir_lowering=False)
    src_k = nc.dram_tensor("src_k", [SRCR, EL], mybir.dt.bfloat16, kind="ExternalInput")
    src_v = nc.dram_tensor("src_v", [SRCR, EL], mybir.dt.bfloat16, kind="ExternalInput")
    gidx = nc.dram_tensor("gidx", [128, L // 16], mybir.dt.int16, kind="ExternalInput")
    posw = nc.dram_tensor("posw", [128, B * L // 128], mybir.dt.int32, kind="ExternalInput")
    pmask = nc.dram_tensor("pmask", [128, B * L // 128], mybir.dt.int32, kind="ExternalInput")
    pdense = nc.dram_tensor("pdense", [128, B * L // 128], mybir.dt.int32, kind="ExternalInput")
    out_k = nc.dram_tensor("out_k", [L, EL], mybir.dt.bfloat16, kind="ExternalOutput")
    out_v = nc.dram_tensor("out_v", [L, EL], mybir.dt.bfloat16, kind="ExternalOutput")
    out_pos = nc.dram_tensor("out_pos", [128, B * L // 128], mybir.dt.int32, kind="ExternalOutput")

    pw = B * L // 128  # 256 int32 per partition
    JB = NIG // 128    # 8 column blocks per gather

    from contextlib import ExitStack

    with ExitStack() as ctx:
        block = ctx.enter_context(nc.Block())
        dst = [
            ctx.enter_context(
                nc.sbuf_tensor(f"dst{i}", [128, JB, EL], mybir.dt.bfloat16)
            )
            for i in range(NBUF)
        ]
        idx_sb = ctx.enter_context(nc.sbuf_tensor("idx_sb", [128, L // 16], mybir.dt.int16))
        pos_sb = ctx.enter_context(nc.sbuf_tensor("pos_sb", [128, pw], mybir.dt.int32))
        pm_sb = ctx.enter_context(nc.sbuf_tensor("pm_sb", [128, pw], mybir.dt.int32))
        pd_sb = ctx.enter_context(nc.sbuf_tensor("pd_sb", [128, pw], mybir.dt.int32))
        po_sb = ctx.enter_context(nc.sbuf_tensor("po_sb", [128, pw], mybir.dt.int32))
        ix = ctx.enter_context(nc.semaphore("ix"))
        io = ctx.enter_context(nc.semaphore("io"))
        pv = ctx.enter_context(nc.semaphore("pv"))
        g_sem = [ctx.enter_context(nc.semaphore(f"g{i}")) for i in range(NBUF)]
        s_sem = [ctx.enter_context(nc.semaphore(f"s{i}")) for i in range(NBUF)]

        NT = 2 * NGATH  # total gather/store pairs (k then v)

        def src_of(t):
            return src_k if t < NGATH else src_v

        def out_view(t):
            g = t % NGATH
            out = out_k if t < NGATH else out_v
            return out[g * NIG : (g + 1) * NIG].rearrange("(j p) e -> p j e", p=128)

        def idx_slice(t):
            g = t % NGATH
            return idx_sb[:, g * (NIG // 16) : (g + 1) * (NIG // 16)]

        @block.sync
        def _(sp):
            sp.dma_start(idx_sb[:], gidx[:]).then_inc(ix, 16)
            sp.dma_start(pos_sb[:], posw[:]).then_inc(io, 16)
            sp.dma_start(pm_sb[:], pmask[:]).then_inc(io, 16)
            sp.dma_start(pd_sb[:], pdense[:]).then_inc(io, 16)
            for t in range(NT):
                b, n = t % NBUF, t // NBUF
                sp.wait_ge(g_sem[b], 16 * (n + 1))
                sp.dma_start(out_view(t), dst[b][:]).then_inc(s_sem[b], 16)
            sp.wait_ge(pv, 1)
            sp.dma_start(out_pos[:], po_sb[:]).then_inc(io, 16)
            sp.wait_ge(io, 64)
            for b in range(NBUF):
                sp.wait_ge(s_sem[b], 16 * (NT // NBUF))

        @block.vector
        def _(ve):
            ve.wait_ge(io, 48)  # pos, mask, dense loaded
            ve.tensor_tensor(
                out=po_sb[:], in0=pos_sb[:], in1=pm_sb[:],
                op=mybir.AluOpType.bitwise_and,
            )
            ve.drain()
            ve.tensor_tensor(
                out=po_sb[:], in0=po_sb[:], in1=pd_sb[:],
                op=mybir.AluOpType.bitwise_or,
            ).then_inc(pv, 1)

        @block.gpsimd
        def _(gp):
            gp.load_library(mlp)
            gp.wait_ge(ix, 16)
            for t in range(NT):
                b, n = t % NBUF, t // NBUF
                if n > 0:
                    gp.wait_ge(s_sem[b], 16 * n)
                gp.dma_gather(
                    dst[b][:], src_of(t)[:], idx_slice(t), NIG, NIG, EL,
                    single_packet=False,
                ).then_inc(g_sem[b], 16)

    nc.compile()
    return nc


def _get_nc():
    global _NC_CACHE
    if _NC_CACHE is None:
        _NC_CACHE = _build_nc()
    return _NC_CACHE


def _host_prep(k_cache, v_cache, pos, fill_indices, input_pos, k_val, v_val):
    fi = np.asarray(fill_indices).astype(np.int64)
    ip = np.asarray(input_pos).astype(np.int32)

    # output row -> row in row-interleaved concat source [cache(L) ; val(S)]
    src_of = np.arange(L, dtype=np.int64)
    src_of[fi] = L + np.arange(S)
    idx_flat = src_of.astype(np.int16)                       # natural row order
    gidx = np.tile(idx_flat.reshape(L // 16, 16).T, (8, 1))  # replicated per Q7 core

    kc = np.asarray(k_cache).reshape(B * H, L, D)
    vc = np.asarray(v_cache).reshape(B * H, L, D)
    kv = np.asarray(k_val).reshape(B * H, S, D)
    vv = np.asarray(v_val).reshape(B * H, S, D)

    posw = np.ascontiguousarray(np.asarray(pos).astype(np.int32)).reshape(128, -1)
    keep = np.full(L, -1, np.int32)
    keep[fi] = 0
    dense = np.zeros(L, np.int32)
    dense[fi] = ip
    pmask = np.tile(keep, B).reshape(128, -1)
    pdense = np.tile(dense, B).reshape(128, -1)

    in_maps = []
    for c in range(N_CORES):
        sl = slice(c * PLANES, (c + 1) * PLANES)
        # (PLANES, rows, D) -> (rows, PLANES, D) -> (rows, EL)
        src_k = np.concatenate(
            [kc[sl].transpose(1, 0, 2), kv[sl].transpose(1, 0, 2)], axis=0
        ).reshape(SRCR, EL)
        src_v = np.concatenate(
            [vc[sl].transpose(1, 0, 2), vv[sl].transpose(1, 0, 2)], axis=0
        ).reshape(SRCR, EL)
        in_maps.append(
            {
                "src_k": np.ascontiguousarray(src_k),
                "src_v": np.ascontiguousarray(src_v),
                "gidx": gidx,
                "posw": posw,
                "pmask": pmask,
                "pdense": pdense,
            }
        )
    return in_maps


def _assemble(res):
    k_parts, v_parts = [], []
    for c in range(N_CORES):
        # (L, EL) -> (L, PLANES, D) -> (PLANES, L, D)
        k_parts.append(res.results[c]["out_k"].reshape(L, PLANES, D).transpose(1, 0, 2))
        v_parts.append(res.results[c]["out_v"].reshape(L, PLANES, D).transpose(1, 0, 2))
    k_new = np.concatenate(k_parts).reshape(B, H, L, D)
    v_new = np.concatenate(v_parts).reshape(B, H, L, D)
    pos_new = res.results[0]["out_pos"].reshape(B, 1, L)
    return k_new, v_new, pos_new


def run(inputs, trace=False):
    """Run the device kernel; returns ((k_new, v_new, pos_new), BassKernelResults)."""
    nc = _get_nc()
    in_maps = _host_prep(**inputs)
    res = run_bass_kernel_spmd(
        nc, in_maps, core_ids=list(range(N_CORES)), trace=trace
    )
    return _assemble(res), res


def kernel(**inputs):
    out, _ = run(inputs, trace=False)
    return out


# revision 7
# speedup vs baseline: 1.1881x; 1.0695x over previous
"""KVCache.fill scatter kernel for Trainium2 (8 NeuronCores, SPMD).

Strategy
--------
The reference scatters k_val/v_val (B,H,S,D) into k_cache/v_cache (B,H,L,D)
at `fill_indices` along the L axis, and input_pos into pos.

Sharding: flatten (B,H) = 128 planes, 16 planes per core (pure data
parallel); fill_indices/input_pos are replicated.

Per plane the output is a row-merge of two sources (cache rows where not
filled, val rows where filled). Because the Bass program is compiled inside
kernel() at call time, the merge pattern is known host-side. The host
stages each core's source in a row-interleaved layout

    src_t[row, plane, d] = concat([cache_rows(L) ; val_rows(S)])

so that ONE dma_gather descriptor (one index) moves the same logical row of
ALL 16 planes: a 4KB contiguous element. The device builds the merged
output with 8 dma_gathers per tensor (1024 indices each, 4KB/descriptor,
HBM->SBUF) + 8 contiguous 4MB stores (SBUF->HBM, 4KB chunks). Descriptor
count per core is 16K (vs 512K at 256B granularity) which keeps the GpSimd
Q7 descriptor-generation off the critical path; every DMA descriptor is
>=4KB so both reads and writes run at line rate.

The device output layout is row-interleaved (L, 16, D); the host transposes
back to (16, L, D) when assembling the full result (pure data staging).

pos (B,1,L) int32 is merged on DVE: pos_new = (pos & keep_mask) | dense,
with keep_mask/dense built host-side from fill_indices/input_pos (index
metadata only). Every core computes it redundantly; core 0's copy is used.
"""

import sys

if "/opt/trn_rl_repo" not in sys.path:
    sys.path.insert(0, "/opt/trn_rl_repo")

import numpy as np

import concourse.bacc as bacc
import concourse.bass as bass
import concourse.mybir as mybir
from concourse.bass_utils import run_bass_kernel_spmd
from concourse.library_config import mlp

B, H, L, D, S = 4, 32, 8192, 128, 2048
N_CORES = 8
PLANES = (B * H) // N_CORES          # 16 planes per core
SRCR = L + S                         # 10240 rows in concat source
EL = PLANES * D                      # 2048 bf16 elements (4KB) per gathered row
NIG = 1024                           # indices per dma_gather
NGATH = L // NIG                     # 8 gathers per tensor
NBUF = 5                             # rotating SBUF tiles

_NC_CACHE = None


def _build_nc():
    nc = bacc.Bacc("TRN2", target_bir_lowering=False)
    src_k = nc.dram_tensor("src_k", [SRCR, EL], mybir.dt.bfloat16, kind="ExternalInput")
    src_v = nc.dram_tensor("src_v", [SRCR, EL], mybir.dt.bfloat16, kind="ExternalInput")
    gidx = nc.dram_tensor("gidx", [128, L // 16], mybir.dt.int16, kind="ExternalInput")
    posw = nc.dram_tensor("posw", [128, B * L // 128], mybir.dt.int32, kind="ExternalInput")
    pmask = nc.dram_tensor("pmask", [128, B * L // 128], mybir.dt.int32, kind="ExternalInput")
    pdense = nc.dram_tensor("pdense", [128, B * L // 128], mybir.dt.int32, kind="ExternalInput")
    out_k = nc.dram_tensor("out_k", [L, EL], mybir.dt.bfloat16, kind="ExternalOutput")
    out_v = nc.dram_tensor("out_v", [L, EL], mybir.dt.bfloat16, kind="ExternalOutput")
    out_pos = nc.dram_tensor("out_pos", [128, B * L // 128], mybir.dt.int32, kind="ExternalOutput")

    pw = B * L // 128  # 256 int32 per partition
    JB = NIG // 128    # 8 column blocks per gather

    from contextlib import ExitStack

    with ExitStack() as ctx:
        block = ctx.enter_context(nc.Block())
        dst = [
            ctx.enter_context(
                nc.sbuf_tensor(f"dst{i}", [128, JB, EL], mybir.dt.bfloat16)
            )
            for i in range(NBUF)
        ]
        idx_sb = ctx.enter_context(nc.sbuf_tensor("idx_sb", [128, L // 16], mybir.dt.int16))
        pos_sb = ctx.enter_context(nc.sbuf_tensor("pos_sb", [128, pw], mybir.dt.int32))
        pm_sb = ctx.enter_context(nc.sbuf_tensor("pm_sb", [128, pw], mybir.dt.int32))
        pd_sb = ctx.enter_context(nc.sbuf_tensor("pd_sb", [128, pw], mybir.dt.int32))
        po_sb = ctx.enter_context(nc.sbuf_tensor("po_sb", [128, pw], mybir.dt.int32))
        ix = ctx.enter_context(nc.semaphore("ix"))
        io = ctx.enter_context(nc.semaphore("io"))
        pv = ctx.enter_context(nc.semaphore("pv"))
        g_sem = [ctx.enter_context(nc.semaphore(f"g{i}")) for i in range(NBUF)]
        s_sem = [ctx.enter_context(nc.semaphore(f"s{i}")) for i in range(NBUF)]

        NT = 2 * NGATH  # total gather/store pairs (k then v)

        def src_of(t):
            return src_k if t < NGATH else src_v

        def out_view(t):
            g = t % NGATH
            out = out_k if t < NGATH else out_v
            return out[g * NIG : (g + 1) * NIG].rearrange("(j p) e -> p j e", p=128)

        def idx_slice(t):
            g = t % NGATH
            return idx_sb[:, g * (NIG // 16) : (g + 1) * (NIG // 16)]

        @block.sync
        def _(sp):
            sp.dma_start(idx_sb[:], gidx[:]).then_inc(ix, 16)
            sp.dma_start(pos_sb[:], posw[:]).then_inc(io, 16)
            sp.dma_start(pm_sb[:], pmask[:]).then_inc(io, 16)
            sp.dma_start(pd_sb[:], pdense[:]).then_inc(io, 16)
            for t in range(NT):
                b, n = t % NBUF, t // NBUF
                sp.wait_ge(g_sem[b], 16 * (n + 1))
                sp.dma_start(out_view(t), dst[b][:]).then_inc(s_sem[b], 16)
            sp.wait_ge(pv, 1)
            sp.dma_start(out_pos[:], po_sb[:]).then_inc(io, 16)
            sp.wait_ge(io, 64)
            for b in range(NBUF):
                sp.wait_ge(s_sem[b], 16 * (NT // NBUF))

        @block.vector
        def _(ve):
            ve.wait_ge(io, 48)  # pos, mask, dense loaded
            ve.tensor_tensor(
                out=po_sb[:], in0=pos_sb[:], in1=pm_sb[:],
                op=mybir.AluOpType.bitwise_and,
            )
            ve.drain()
            ve.tensor_tensor(
                out=po_sb[:], in0=po_sb[:], in1=pd_sb[:],
                op=mybir.AluOpType.bitwise_or,
            ).then_inc(pv, 1)

        @block.gpsimd
        def _(gp):
            gp.load_library(mlp)
            gp.wait_ge(ix, 16)
            for t in range(NT):
                b, n = t % NBUF, t // NBUF
                if n > 0:
                    gp.wait_ge(s_sem[b], 16 * n)
                gp.dma_gather(
                    dst[b][:], src_of(t)[:], idx_slice(t), NIG, NIG, EL,
                    single_packet=False,
                ).then_inc(g_sem[b], 16)

    nc.compile()
    return nc


def _get_nc():
    global _NC_CACHE
    if _NC_CACHE is None:
        _NC_CACHE = _build_nc()
    return _NC_CACHE


def _host_prep(k_cache, v_cache, pos, fill_indices, input_pos, k_val, v_val):
    fi = np.asarray(fill_indices).astype(np.int64)
    ip = np.asarray(input_pos).astype(np.int32)

    # output row -> row in row-interleaved concat source [cache(L) ; val(S)]
    src_of = np.arange(L, dtype=np.int64)
    src_of[fi] = L + np.arange(S)
    idx_flat = src_of.astype(np.int16)                       # natural row order
    gidx = np.tile(idx_flat.reshape(L // 16, 16).T, (8, 1))  # replicated per Q7 core

    kc = np.asarray(k_cache).reshape(B * H, L, D)
    vc = np.asarray(v_cache).reshape(B * H, L, D)
    kv = np.asarray(k_val).reshape(B * H, S, D)
    vv = np.asarray(v_val).reshape(B * H, S, D)

    posw = np.ascontiguousarray(np.asarray(pos).astype(np.int32)).reshape(128, -1)
    keep = np.full(L, -1, np.int32)
    keep[fi] = 0
    dense = np.zeros(L, np.int32)
    dense[fi] = ip
    pmask = np.tile(keep, B).reshape(128, -1)
    pdense = np.tile(dense, B).reshape(128, -1)

    in_maps = []
    for c in range(N_CORES):
        sl = slice(c * PLANES, (c + 1) * PLANES)
        # (PLANES, rows, D) -> (rows, PLANES, D) -> (rows, EL)
        src_k = np.concatenate(
            [kc[sl].transpose(1, 0, 2), kv[sl].transpose(1, 0, 2)], axis=0
        ).reshape(SRCR, EL)
        src_v = np.concatenate(
            [vc[sl].transpose(1, 0, 2), vv[sl].transpose(1, 0, 2)], axis=0
        ).reshape(SRCR, EL)
        in_maps.append(
            {
                "src_k": np.ascontiguousarray(src_k),
                "src_v": np.ascontiguousarray(src_v),
                "gidx": gidx,
                "posw": posw,
                "pmask": pmask,
                "pdense": pdense,
            }
        )
    return in_maps


def _assemble(res):
    k_parts, v_parts = [], []
    for c in range(N_CORES):
        # (L, EL) -> (L, PLANES, D) -> (PLANES, L, D)
        k_parts.append(res.results[c]["out_k"].reshape(L, PLANES, D).transpose(1, 0, 2))
        v_parts.append(res.results[c]["out_v"].reshape(L, PLANES, D).transpose(1, 0, 2))
    k_new = np.concatenate(k_parts).reshape(B, H, L, D)
    v_new = np.concatenate(v_parts).reshape(B, H, L, D)
    pos_new = res.results[0]["out_pos"].reshape(B, 1, L)
    return k_new, v_new, pos_new


def run(inputs, trace=False):
    """Run the device kernel; returns ((k_new, v_new, pos_new), BassKernelResults)."""
    nc = _get_nc()
    in_maps = _host_prep(**inputs)
    res = run_bass_kernel_spmd(
        nc, in_maps, core_ids=list(range(N_CORES)), trace=trace
    )
    return _assemble(res), res


def kernel(**inputs):
    out, _ = run(inputs, trace=False)
    return out
